# revision 8
# baseline (speedup 1.0000x reference)
"""Self-contained TRN2 Bass kernel for nn_FLoRALayer (B=8, S=2048, D=1024, R=8).

kernel(**inputs) takes FULL unsharded inputs:
    x         [8, 2048, 1024] f32
    adapter_b [8, 1024, 8]    f32
    adapter_a [8, 8, 1024]    f32
    W0        [1024, 1024]    f32
returns the FULL [8, 2048, 1024] f32 output of:
    BxW0 = einsum('bsd,bdr,do->bsro', x.astype(fp16), adapter_b, W0)
    out  = relu(mean(swapaxes(adapter_a,1,2)[:,None]*BxW0.reshape(b,s,d,r), -1))

Math refactor (verified exactly): with o = kk*128 + g*16 + mp,
    W_eff[dd, o] = adapter_b[dd, kk] * sum_rp adapter_a[rp, o] * W0[dd, (o%128)*8 + rp]
    out[b] = relu((x[b] @ W_eff[b]) / 8)
one [2048,1024] @ [1024,1024] matmul per batch; batch b runs on NeuronCore b.

v2 design (vs the 95us baseline):
  - weff kept in PSUM-native column order j = g*128+kk*16+mp; host unpermutes
    the output columns (pure data movement) so the BTT is fully contiguous.
  - adapter uploads are compact (aspc 32KB, bcp 32KB + an 8KB constant 0/1
    mask); the block-diagonal asp operand is built on-device with one DVE
    multiply (aspc broadcast over mp) x (mask broadcast over g,kk).
  - main matmuls run f32r x f32r (bitcast of f32 data, 1 row/cycle at N=512
    per the cost model): no x casts at all, x tiles feed the PE directly
    from their DMA. weff is built f32 by the BTT (psum f32 * bcp f32).
  - the small C matmuls stay fp16 (N=128 would be 4x slower in f32r): W0
    tiles are cast f32->fp16 on the ACT engine (off the DVE critical path).
  - deep prefetch: all 16 x-tile DMAs issued upfront on the sync ring, all
    8 W0 tiles + tiny consts on the gpsimd ring; output stores go on the
    scalar/gpsimd rings which are drained early, so stores flow promptly
    and PSUM/outst slots recycle without stalling the PE.
"""

import os
from contextlib import ExitStack

import numpy as np

S, D, R = 2048, 1024, 8
NT = D // 128
NS = S // 128
WARM = [0, 1, 2]
N_CORES = 8

DEFAULT_MODE = os.environ.get("FLORA_MODE", "f32r")

_compiled = {}


def _build_kernel(mode=DEFAULT_MODE):
    import concourse.bass as bass
    import concourse.tile as tile
    from concourse import bacc, mybir

    F32 = mybir.dt.float32
    F16 = mybir.dt.float16
    F32R = mybir.dt.float32r

    nc = bacc.Bacc(
        "TRN2", target_bir_lowering=False, debug=False, num_devices=N_CORES
    )

    xdt = F32R if mode == "f32r" else F32
    x_d = nc.dram_tensor("xtp", [NS, 128, D], xdt, kind="ExternalInput").ap()
    w0_d = nc.dram_tensor("w0tp", [NT, 128, D], F32, kind="ExternalInput").ap()
    aspc_d = nc.dram_tensor("aspc", [128, 64], F32, kind="ExternalInput").ap()
    mask_d = nc.dram_tensor("mmask", [128, 16], F32, kind="ExternalInput").ap()
    bcp_d = nc.dram_tensor("bcp", [128, 64], F32, kind="ExternalInput").ap()
    out_d = nc.dram_tensor("out", [S, D], F32, kind="ExternalOutput").ap()

    wdt = F32R if mode == "f32r" else F16

    with tile.TileContext(nc) as tc, ExitStack() as ctx:
        pool = lambda name, bufs, **kw: ctx.enter_context(
            tc.tile_pool(name=name, bufs=bufs, **kw)
        )
        const_p = pool("const", 1)
        big_p = pool("big", 1)
        outst_p = pool("outst", 5)
        pmm_p = pool("pmm", 1, space="PSUM")

        xall = big_p.tile([128, NS * D], xdt, tag="xall")
        w0all = big_p.tile([128, NT * D], F32, tag="w0all")
        w0h = big_p.tile([128, NT * D], F16, tag="w0h")
        weff = big_p.tile([128, NT * D], wdt, tag="weff")
        aspc = const_p.tile([128, 64], F32, tag="aspc")
        mmask = const_p.tile([128, 16], F32, tag="mmask")
        bcp = const_p.tile([128, 64], F32, tag="bcp")
        asph = const_p.tile([128, NT * 128], F16, tag="asph")

        if mode == "fp16":
            xth = big_p.tile([128, NS * D], F16, tag="xth")

        # ---- DMA issues (order within each ring = transfer order) ----
        nc.gpsimd.dma_start(aspc[:], aspc_d[:])
        nc.gpsimd.dma_start(mmask[:], mask_d[:])
        nc.gpsimd.dma_start(bcp[:], bcp_d[:])
        for t in range(NT):
            nc.gpsimd.dma_start(w0all[:, t * D : (t + 1) * D], w0_d[t])
        for s in range(NS):
            nc.sync.dma_start(xall[:, s * D : (s + 1) * D], x_d[s])

        # ---- asp scatter: asph[p, g*128+kk*16+mp] = aspc[p, g*8+kk]*mask[p,mp]
        nc.vector.tensor_tensor(
            out=asph[:].rearrange("p (g kk mp) -> p g kk mp", g=8, kk=8),
            in0=aspc[:]
            .rearrange("p (g kk) -> p g kk", g=8)[:, :, :, None]
            .broadcast_to([128, 8, 8, 16]),
            in1=mmask[:][:, None, None, :].broadcast_to([128, 8, 8, 16]),
            op=mybir.AluOpType.mult,
        )

        def x_op(s):
            if mode == "f32r":
                return xall[:, s * D : (s + 1) * D]
            return xth[:, s * D : (s + 1) * D]

        def weff_op(c, h):
            return weff[:, c * D + h * 512 : c * D + (h + 1) * 512]

        xc_done = set()

        def x_cast(s):
            if mode == "fp16" and s < NS and s not in xc_done:
                xc_done.add(s)
                nc.vector.tensor_copy(
                    xth[:, s * D : (s + 1) * D], xall[:, s * D : (s + 1) * D]
                )

        if mode == "fp16":
            for s in WARM:
                x_cast(s)

        po_warm = {
            s: [
                pmm_p.tile([128, 512], F32, tag="po", bufs=6, name=f"po{s}_{i}")
                for i in range(2)
            ]
            for s in WARM
        }

        def warm_mms(c):
            for s in WARM:
                xop = x_op(s)
                for h in range(2):
                    nc.tensor.matmul(
                        po_warm[s][h][:],
                        lhsT=xop[:, c * 128 : (c + 1) * 128],
                        rhs=weff_op(c, h),
                        start=(c == 0),
                        stop=(c == NT - 1),
                    )

        # ---- weff chain: per t: cast w0 (ACT) -> 8 C matmuls -> BTT ----
        for t in range(NT):
            nc.scalar.activation(
                w0h[:, t * D : (t + 1) * D],
                w0all[:, t * D : (t + 1) * D],
                mybir.ActivationFunctionType.Copy,
                scale=1.0,
            )
            pcs = [
                pmm_p.tile([128, 512], F32, tag="pc", bufs=2, name=f"pc{t}_{i}")
                for i in range(2)
            ]
            for g in range(NT):
                nc.tensor.matmul(
                    pcs[g // 4][:, (g % 4) * 128 : (g % 4 + 1) * 128],
                    lhsT=w0h[:, t * D + g * 128 : t * D + (g + 1) * 128],
                    rhs=asph[:, g * 128 : (g + 1) * 128],
                    start=True,
                    stop=True,
                )
            for h in range(2):
                wv = weff[
                    :, t * D + h * 512 : t * D + (h + 1) * 512
                ].rearrange("p (g kk mp) -> p g kk mp", g=4, kk=8)
                pv = pcs[h][:].rearrange("p (g kk mp) -> p g kk mp", g=4, kk=8)
                bv = bcp[:, t * 8 : (t + 1) * 8][:, None, :, None].broadcast_to(
                    [128, 4, 8, 16]
                )
                nc.vector.tensor_tensor(
                    out=wv, in0=pv, in1=bv, op=mybir.AluOpType.mult
                )
            x_cast(3 + t)
            # consume weff one chunk behind its construction: warm(t-1) only
            # needs BTT(t-1), which finished during C(t) -- no serial chain
            if t > 0:
                warm_mms(t - 1)
        warm_mms(NT - 1)

        # ---- steady mains + evac/store ----
        for s in range(NS):
            if s not in WARM:
                x_cast(s + 1)
                x_cast(s + 2)
                xop = x_op(s)
                po = [
                    pmm_p.tile(
                        [128, 512], F32, tag="po", bufs=6, name=f"po{s}_{i}"
                    )
                    for i in range(2)
                ]
                for c in range(NT):
                    for h in range(2):
                        nc.tensor.matmul(
                            po[h][:],
                            lhsT=xop[:, c * 128 : (c + 1) * 128],
                            rhs=weff_op(c, h),
                            start=(c == 0),
                            stop=(c == NT - 1),
                        )
            else:
                po = po_warm[s]
            outst = outst_p.tile([128, D], F32, tag="outst", name=f"outst{s}")
            nc.scalar.activation(
                outst[:, 0:512],
                po[0][:],
                mybir.ActivationFunctionType.Relu,
                scale=0.125,
            )
            nc.scalar.dma_start(
                out_d[s * 128 : (s + 1) * 128, 0:512], outst[:, 0:512]
            )
            nc.vector.tensor_scalar(
                out=outst[:, 512:1024],
                in0=po[1][:],
                scalar1=0.125,
                scalar2=0.0,
                op0=mybir.AluOpType.mult,
                op1=mybir.AluOpType.max,
            )
            nc.gpsimd.dma_start(
                out_d[s * 128 : (s + 1) * 128, 512:1024], outst[:, 512:1024]
            )

    nc.compile()
    return nc


# output column unpermute: stored col j=(g,kk,mp) -> true col o=kk*128+g*16+mp
_j = np.arange(D)
_PERM = ((_j >> 4) & 7) * 128 + (_j >> 7) * 16 + (_j & 15)
_INV = np.argsort(_PERM)  # out_full[..., o] = stored[..., _INV[o]]


def _pack_inputs(x_b, adapter_b_b, adapter_a_b, W0):
    """Pure data placement (permutation / replication / zero-padding)."""
    xtp = np.ascontiguousarray(
        x_b.reshape(NS, 128, NT, 128).transpose(0, 3, 2, 1).reshape(NS, 128, D),
        np.float32,
    )
    w0tp = np.ascontiguousarray(
        W0.reshape(NT, 128, NT, 128).transpose(0, 3, 2, 1).reshape(NT, 128, D),
        np.float32,
    )
    # aspc[mp*8+rp, g*8+kk] = a[rp, kk*128+g*16+mp]
    mp_i, rp_i, g_i, kk_i = np.meshgrid(
        np.arange(16), np.arange(8), np.arange(8), np.arange(8), indexing="ij"
    )
    aspc = np.ascontiguousarray(
        adapter_a_b[rp_i, kk_i * 128 + g_i * 16 + mp_i].reshape(128, 64),
        np.float32,
    )
    mmask = (
        np.arange(16)[None, :] == (np.arange(128)[:, None] // 8)
    ).astype(np.float32)
    # bcp[dp, t*8+kk] = b[t*128+dp, kk]
    bcp = np.ascontiguousarray(
        adapter_b_b.reshape(NT, 128, R).transpose(1, 0, 2).reshape(128, NT * R),
        np.float32,
    )
    return {"xtp": xtp, "w0tp": w0tp, "aspc": aspc, "mmask": mmask, "bcp": bcp}


def kernel(x, adapter_b, adapter_a, W0):
    x = np.asarray(x, np.float32)
    adapter_b = np.asarray(adapter_b, np.float32)
    adapter_a = np.asarray(adapter_a, np.float32)
    W0 = np.asarray(W0, np.float32)
    B = x.shape[0]
    assert B == N_CORES and x.shape == (B, S, D)

    if DEFAULT_MODE not in _compiled:
        _compiled[DEFAULT_MODE] = _build_kernel(DEFAULT_MODE)

    from concourse.bass_utils import run_bass_kernel_spmd

    in_maps = [
        _pack_inputs(x[b], adapter_b[b], adapter_a[b], W0) for b in range(B)
    ]
    res = run_bass_kernel_spmd(_compiled[DEFAULT_MODE], in_maps, list(range(N_CORES)))
    out = np.stack([res.results[b]["out"] for b in range(B)]).astype(np.float32)
    return out[:, :, _INV]


# revision 14
# speedup vs baseline: 1.1167x; 1.1167x over previous
"""Self-contained TRN2 Bass kernel for nn_FLoRALayer (B=8, S=2048, D=1024, R=8).

kernel(**inputs) takes FULL unsharded inputs:
    x         [8, 2048, 1024] f32
    adapter_b [8, 1024, 8]    f32
    adapter_a [8, 8, 1024]    f32
    W0        [1024, 1024]    f32
returns the FULL [8, 2048, 1024] f32 output of:
    BxW0 = einsum('bsd,bdr,do->bsro', x.astype(fp16), adapter_b, W0)
    out  = relu(mean(swapaxes(adapter_a,1,2)[:,None]*BxW0.reshape(b,s,d,r), -1))

Math refactor (verified exactly): with o = kk*128 + g*16 + mp,
    W_eff[dd, o] = adapter_b[dd, kk] * sum_rp adapter_a[rp, o] * W0[dd, (o%128)*8 + rp]
    out[b] = relu((x_fp16[b] @ W_eff[b]) / 8)
one [2048,1024] @ [1024,1024] matmul per batch; batch b runs on NeuronCore b.

v3 schedule (from trace analysis of the 88-95us baseline; PE floor ~59us,
HBM floor ~56us -- a true "ridge" kernel, so the whole job is overlap):
  - weff kept in PSUM-native column order j = g*128+kk*16+mp; host unpermutes
    the output columns (pure data movement), making the BTT fully contiguous.
  - adapter uploads are compact (aspc/bcp 32KB + an 8KB 0/1 mask); the
    block-diagonal asp matmul operand is built on-device with one DVE
    multiply: (aspc broadcast over mp) x (mask broadcast over g,kk).
  - ALL input reads ride ONE dma ring (sync) in priority order
    consts, w0_0, x0, w0_1, x1, w0_2, x2, w0_3, x3, w0_4..w0_7, x4..x15
    so the weff chain (which gates every non-warm main matmul) is paced at
    full HBM bandwidth, while enough x tiles arrive to keep the PE busy.
    Output stores ride the otherwise-idle scalar/gpsimd rings, so they
    never queue behind reads and PSUM/outst slots recycle promptly.
  - PSUM = 7 "po" half-banks (warm tiles s0-s2 both halves + s3 half0,
    cycling in steady state) + 1 "pc" bank for the C matmuls; the C chunk
    runs as two 4-matmul halves with warm mains woven between them.
  - w0 casts split: half on ACT, half on DVE; BTTs on the Pool engine;
    x casts on DVE two tiles ahead of consumption.
"""

import os
from contextlib import ExitStack

import numpy as np

S, D, R = 2048, 1024, 8
NT = D // 128
NS = S // 128
N_CORES = 8

DEFAULT_MODE = os.environ.get("FLORA_MODE", "fp16")

_compiled = {}


def _build_kernel(mode=DEFAULT_MODE):
    import concourse.bass as bass
    import concourse.tile as tile
    from concourse import bacc, mybir

    F32 = mybir.dt.float32
    F16 = mybir.dt.float16
    F32R = mybir.dt.float32r

    nc = bacc.Bacc(
        "TRN2", target_bir_lowering=False, debug=False, num_devices=N_CORES
    )

    xdt = F32R if mode == "f32r" else F32
    wdt = F32R if mode == "f32r" else F16
    x_d = nc.dram_tensor("xtp", [NS, 128, D], xdt, kind="ExternalInput").ap()
    w0_d = nc.dram_tensor("w0tp", [NT, 128, D], F32, kind="ExternalInput").ap()
    aspc_d = nc.dram_tensor("aspc", [128, 64], F32, kind="ExternalInput").ap()
    mask_d = nc.dram_tensor("mmask", [128, 16], F32, kind="ExternalInput").ap()
    bcp_d = nc.dram_tensor("bcp", [128, 64], F32, kind="ExternalInput").ap()
    out_d = nc.dram_tensor("out", [S, D], F32, kind="ExternalOutput").ap()

    with tile.TileContext(nc) as tc, ExitStack() as ctx:
        pool = lambda name, bufs, **kw: ctx.enter_context(
            tc.tile_pool(name=name, bufs=bufs, **kw)
        )
        const_p = pool("const", 1)
        big_p = pool("big", 1)
        outst_p = pool("outst", 5)
        pmm_p = pool("pmm", 1, space="PSUM")

        xall = big_p.tile([128, NS * D], xdt, tag="xall")
        w0all = big_p.tile([128, NT * D], F32, tag="w0all")
        w0h = big_p.tile([128, NT * D], F16, tag="w0h")
        weff = big_p.tile([128, NT * D], wdt, tag="weff")
        aspc = const_p.tile([128, 64], F32, tag="aspc")
        mmask = const_p.tile([128, 16], F32, tag="mmask")
        bcp = const_p.tile([128, 64], F32, tag="bcp")
        asph = const_p.tile([128, NT * 128], F16, tag="asph")
        if mode == "fp16":
            xth = big_p.tile([128, NS * D], F16, tag="xth")

        # ---- all input reads on the sync ring, in priority order ----
        nc.sync.dma_start(aspc[:], aspc_d[:])
        nc.sync.dma_start(mmask[:], mask_d[:])
        nc.sync.dma_start(bcp[:], bcp_d[:])

        def w0_dma(t):
            nc.sync.dma_start(w0all[:, t * D : (t + 1) * D], w0_d[t])

        def x_dma(s):
            nc.sync.dma_start(xall[:, s * D : (s + 1) * D], x_d[s])

        for u in range(4):  # w0_0,x0,w0_1,x1,w0_2,x2,w0_3,x3
            w0_dma(u)
            x_dma(u)
        for t in range(4, NT):  # w0_4..w0_7 back to back
            w0_dma(t)
        for s in range(4, NS):  # x4..x15
            x_dma(s)

        # ---- asp scatter: asph[p, g*128+kk*16+mp] = aspc[p,g*8+kk]*mask[p,mp]
        nc.vector.tensor_tensor(
            out=asph[:].rearrange("p (g kk mp) -> p g kk mp", g=8, kk=8),
            in0=aspc[:]
            .rearrange("p (g kk) -> p g kk", g=8)[:, :, :, None]
            .broadcast_to([128, 8, 8, 16]),
            in1=mmask[:][:, None, None, :].broadcast_to([128, 8, 8, 16]),
            op=mybir.AluOpType.mult,
        )

        def x_op(s):
            if mode == "f32r":
                return xall[:, s * D : (s + 1) * D]
            return xth[:, s * D : (s + 1) * D]

        def weff_op(c, h):
            return weff[:, c * D + h * 512 : c * D + (h + 1) * 512]

        xc_done = set()

        def x_cast(s):
            if mode == "fp16" and 0 <= s < NS and s not in xc_done:
                xc_done.add(s)
                nc.vector.tensor_copy(
                    xth[:, s * D : (s + 1) * D], xall[:, s * D : (s + 1) * D]
                )

        for s in range(3):
            x_cast(s)

        # warm units: (s, h) psum halves in flight during the weff chain
        WARMH = [(0, 0), (0, 1), (1, 0), (1, 1), (2, 0), (2, 1), (3, 0)]
        po_warm = {
            (s, h): pmm_p.tile(
                [128, 512], F32, tag="po", bufs=7, name=f"po{s}_{h}"
            )
            for (s, h) in WARMH
        }

        def warm_unit(s, h, c):
            nc.tensor.matmul(
                po_warm[(s, h)][:],
                lhsT=x_op(s)[:, c * 128 : (c + 1) * 128],
                rhs=weff_op(c, h),
                start=(c == 0),
                stop=(c == NT - 1),
            )

        def warm_block(t, part):
            # chunk t-1 for s0-s2; s3h0 catches up 2 chunks per step t=4..7
            c = t - 1
            if part == 0:
                for s, h in WARMH[:4]:
                    warm_unit(s, h, c)
            else:
                for s, h in WARMH[4:6]:
                    warm_unit(s, h, c)
                if 4 <= t <= 7:
                    for cc in (2 * (t - 4), 2 * (t - 4) + 1):
                        warm_unit(3, 0, cc)

        # ---- weff chain ----
        for t in range(NT):
            # cast w0 tile t: half0 on ACT, half1 on DVE
            nc.scalar.activation(
                w0h[:, t * D : t * D + 512],
                w0all[:, t * D : t * D + 512],
                mybir.ActivationFunctionType.Copy,
                scale=1.0,
            )
            nc.gpsimd.tensor_copy(
                w0h[:, t * D + 512 : (t + 1) * D],
                w0all[:, t * D + 512 : (t + 1) * D],
            )
            if t == 3:
                x_cast(3)
            for half in range(2):
                pc = pmm_p.tile(
                    [128, 512], F32, tag="pc", bufs=1, name=f"pc{t}_{half}"
                )
                for gg in range(4):
                    g = half * 4 + gg
                    nc.tensor.matmul(
                        pc[:, gg * 128 : (gg + 1) * 128],
                        lhsT=w0h[:, t * D + g * 128 : t * D + (g + 1) * 128],
                        rhs=asph[:, g * 128 : (g + 1) * 128],
                        start=True,
                        stop=True,
                    )
                wv = weff[
                    :, t * D + half * 512 : t * D + (half + 1) * 512
                ].rearrange("p (g kk mp) -> p g kk mp", g=4, kk=8)
                pv = pc[:].rearrange("p (g kk mp) -> p g kk mp", g=4, kk=8)
                bv = bcp[:, t * 8 : (t + 1) * 8][:, None, :, None].broadcast_to(
                    [128, 4, 8, 16]
                )
                nc.vector.tensor_tensor(
                    out=wv, in0=pv, in1=bv, op=mybir.AluOpType.mult
                )
                if t > 0:
                    warm_block(t, half)
        warm_block(NT, 0)
        warm_block(NT, 1)

        # warm tiles s0-s2: evac as soon as the chain completes
        for s in range(3):
            self_evac(
                nc, mybir, out_d, outst_p,
                [po_warm[(s, 0)], po_warm[(s, 1)]], s,
            )

        # ---- steady mains + evac/store ----
        for s in range(3, NS):
            x_cast(s + 1)
            x_cast(s + 2)
            if s == 3:
                po = [po_warm[(3, 0)], None]
                hs = [1]
            else:
                po = [None, None]
                hs = [0, 1]
            for h in hs:
                po[h] = pmm_p.tile(
                    [128, 512], F32, tag="po", bufs=7, name=f"po{s}_{h}"
                )
            for c in range(NT):
                for h in hs:
                    nc.tensor.matmul(
                        po[h][:],
                        lhsT=x_op(s)[:, c * 128 : (c + 1) * 128],
                        rhs=weff_op(c, h),
                        start=(c == 0),
                        stop=(c == NT - 1),
                    )
            self_evac(nc, mybir, out_d, outst_p, po, s)

    nc.compile()
    return nc


def self_evac(nc, mybir, out_d, outst_p, po, s):
    outst = outst_p.tile([128, D], mybir.dt.float32, tag="outst", name=f"o{s}")
    nc.scalar.activation(
        outst[:, 0:512],
        po[0][:],
        mybir.ActivationFunctionType.Relu,
        scale=0.125,
    )
    nc.scalar.dma_start(out_d[s * 128 : (s + 1) * 128, 0:512], outst[:, 0:512])
    nc.vector.tensor_scalar(
        out=outst[:, 512:1024],
        in0=po[1][:],
        scalar1=0.125,
        scalar2=0.0,
        op0=mybir.AluOpType.mult,
        op1=mybir.AluOpType.max,
    )
    nc.gpsimd.dma_start(
        out_d[s * 128 : (s + 1) * 128, 512:1024], outst[:, 512:1024]
    )


# output column unpermute: stored col j=(g,kk,mp) -> true col o=kk*128+g*16+mp
_j = np.arange(D)
_PERM = ((_j >> 4) & 7) * 128 + (_j >> 7) * 16 + (_j & 15)
_INV = np.argsort(_PERM)  # out_full[..., o] = stored[..., _INV[o]]


def _pack_inputs(x_b, adapter_b_b, adapter_a_b, W0):
    """Pure data placement (permutation / replication / zero-padding)."""
    xtp = np.ascontiguousarray(
        x_b.reshape(NS, 128, NT, 128).transpose(0, 3, 2, 1).reshape(NS, 128, D),
        np.float32,
    )
    w0tp = np.ascontiguousarray(
        W0.reshape(NT, 128, NT, 128).transpose(0, 3, 2, 1).reshape(NT, 128, D),
        np.float32,
    )
    # aspc[mp*8+rp, g*8+kk] = a[rp, kk*128+g*16+mp]
    mp_i, rp_i, g_i, kk_i = np.meshgrid(
        np.arange(16), np.arange(8), np.arange(8), np.arange(8), indexing="ij"
    )
    aspc = np.ascontiguousarray(
        adapter_a_b[rp_i, kk_i * 128 + g_i * 16 + mp_i].reshape(128, 64),
        np.float32,
    )
    mmask = (
        np.arange(16)[None, :] == (np.arange(128)[:, None] // 8)
    ).astype(np.float32)
    # bcp[dp, t*8+kk] = b[t*128+dp, kk]
    bcp = np.ascontiguousarray(
        adapter_b_b.reshape(NT, 128, R).transpose(1, 0, 2).reshape(128, NT * R),
        np.float32,
    )
    return {"xtp": xtp, "w0tp": w0tp, "aspc": aspc, "mmask": mmask, "bcp": bcp}


def kernel(x, adapter_b, adapter_a, W0):
    x = np.asarray(x, np.float32)
    adapter_b = np.asarray(adapter_b, np.float32)
    adapter_a = np.asarray(adapter_a, np.float32)
    W0 = np.asarray(W0, np.float32)
    B = x.shape[0]
    assert B == N_CORES and x.shape == (B, S, D)

    if DEFAULT_MODE not in _compiled:
        _compiled[DEFAULT_MODE] = _build_kernel(DEFAULT_MODE)

    from concourse.bass_utils import run_bass_kernel_spmd

    in_maps = [
        _pack_inputs(x[b], adapter_b[b], adapter_a[b], W0) for b in range(B)
    ]
    res = run_bass_kernel_spmd(
        _compiled[DEFAULT_MODE], in_maps, list(range(N_CORES))
    )
    out = np.stack([res.results[b]["out"] for b in range(B)]).astype(np.float32)
    return out[:, :, _INV]
